# revision 50
# baseline (speedup 1.0000x reference)
"""DynamicLoRAConv1d kernel for 8 Trainium2 NeuronCores.

Math: the per-sample LoRA conv is linear in weights, so
  conv(x, W) + conv(x, dW_b) = conv(x, W + dW_b)
with dW_b = lora_scale * (B_b @ A_b).  The tiny per-sample effective weight
(conv_w + dW_b) is fused on host.  Host prep also deinterleaves the padded
input on the time axis (even positions -> partitions 0..63, odd -> 64..127,
bf16, image-inner DRAM layout), so conv tap pairs (2m, 2m+1) fuse into
K=128 unit-stride matmuls: 3 matmuls of N=1024 per image (taps (0,1),
(2,3) at K=128, tap 4 at K=64) accumulated into one 2-bank PSUM tile.

Per image:
  conv:  3 matmuls (N=1024) -> PSUM [128,1024] f32.
  relu:  one ScalarE ACT: y = relu(ps + bias) -> bf16, with accum_out
         giving per-channel sum(z) for free.
  zsq:   one DVE tensor_tensor_reduce: zq = y*y (scratch) with accum_out
         giving per-channel sum(z^2).
Per block of 8 images (GroupNorm stats batched):
  sums/sumsq land in a [128, 16] block-stats tile; one DVE 32x32
  block-transpose -> free-dim reduce -> broadcast -> transpose back
  produces per-channel group sums; a 7-op chain on [128,8] tiles yields
  per-channel scale/offset for all 8 images at once.
Apply (per image, lagged 8 behind):
  out = y*scale+offset, split DVE (cols 0:512) / GpSimd (512:1024),
  fp16, into a 4-image output tile; one DMA out per 4 images.

DMA: input batched 4 images per transfer (sync queue), output 4 images
per transfer; cuts HWDGE issue serialization 4x.

Sharding: data-parallel over Batch - core c gets samples 4c..4c+3
(= images 32c..32c+32).  No cross-core communication.
"""

import sys
from contextlib import ExitStack

import numpy as np

for _p in ("/opt/trn_rl_repo", "/opt/pypackages"):
    if _p not in sys.path:
        sys.path.append(_p)

import concourse.bacc as bacc
import concourse.bass as bass
import concourse.mybir as mybir
import concourse.tile as tile
from concourse.bass_utils import run_bass_kernel_spmd

F32 = mybir.dt.float32
BF16 = mybir.dt.bfloat16
FP16 = mybir.dt.float16
AF = mybir.ActivationFunctionType
ALU = mybir.AluOpType

N_CORES = 8
SAMPLES = 4      # samples per core
SENSORS = 8
IMGS = SAMPLES * SENSORS  # images per core
IN_C = 64
OUT_C = 128
KTAPS = 5
T = 2048
T_PAD = T + 4    # 2052
T_HALF = T_PAD // 2  # 1026 deinterleaved columns
T_OUT = 1024
EPS = 1e-5
G = 4
CPG = OUT_C // G
BLK = 4          # images per GroupNorm-stats block
QB = 4           # images per DMA batch
MTOT = float(CPG * T_OUT)  # 32768 elements per group
LAG = 8          # apply lags conv by this many images

TRACE = False
LAST_RESULTS = None

_PROGRAM = None


def _build_program():
    nc = bacc.Bacc("TRN2", target_bir_lowering=False, debug=False)
    xin = nc.dram_tensor("xin", [2 * IN_C, IMGS, T_HALF], BF16,
                         kind="ExternalInput")
    wts = nc.dram_tensor("wts", [2 * IN_C, SAMPLES * 3 * OUT_C], BF16,
                         kind="ExternalInput")
    cons = nc.dram_tensor("cons", [OUT_C, 4], F32, kind="ExternalInput")
    out = nc.dram_tensor("out", [OUT_C, IMGS, T_OUT], FP16,
                         kind="ExternalOutput")

    with ExitStack() as ctx:
        tc = ctx.enter_context(tile.TileContext(nc))
        cpool = ctx.enter_context(tc.tile_pool(name="cpool", bufs=1))
        xpool = ctx.enter_context(tc.tile_pool(name="xpool", bufs=3))
        ypool = ctx.enter_context(tc.tile_pool(name="ypool", bufs=7))
        zpool = ctx.enter_context(tc.tile_pool(name="zpool", bufs=3))
        opool = ctx.enter_context(tc.tile_pool(name="opool", bufs=3))
        spool = ctx.enter_context(tc.tile_pool(name="spool", bufs=4))
        tpool = ctx.enter_context(tc.tile_pool(name="tpool", bufs=4))
        pspool = ctx.enter_context(tc.tile_pool(name="pspool", bufs=2,
                                                space="PSUM"))

        # ---- persistent constants ----
        # weights/consts ride the DVE queue so they overlap the first
        # image DMAs on the SP queue
        wt = cpool.tile([2 * IN_C, SAMPLES * 3 * OUT_C], BF16)
        nc.scalar.dma_start(out=wt[:], in_=wts.ap()[:])
        ct = cpool.tile([OUT_C, 4], F32)
        bias_ap = ct[:, 0:1]    # conv bias
        mgam_ap = ct[:, 1:2]    # MTOT * gamma
        beta_ap = ct[:, 2:3]    # beta
        m2eps_ap = ct[:, 3:4]   # MTOT^2 * eps
        ones32 = cpool.tile([OUT_C, 32], F32)
        nc.gpsimd.memset(ones32[:], 1.0)

        st = {}
        blk = {}

        # PE pre-warm: dummy matmuls on zeroed scratch while the first
        # input DMAs are in flight, so the PE's DVFS p-state is ramped
        # when the real convs start.  They write into image 0's PSUM tile
        # (overwritten by the real conv's start=True).
        pwarm = cpool.tile([2 * IN_C, 640], BF16)
        nc.gpsimd.memset(pwarm[:], 0.0)
        ps0 = pspool.tile([OUT_C, 2 * T_OUT], F32, tag="ps2", name="ps2_0")
        st[("ps2", 0)] = ps0
        for _k in range(10):
            nc.tensor.matmul(ps0[:, 0:512], pwarm[:, 512:640],
                             pwarm[:, 0:512], start=True, stop=True)

        def in_dma(p):
            """Image-pair input DMA into half of the batch tile; subtile
            dep tracking lets conv_pair(2p) start as soon as its own pair
            lands."""
            j = 2 * p
            q, jq = j // QB, j % QB
            if jq == 0:
                st[("x", q)] = xpool.tile([2 * IN_C, QB * T_HALF], BF16,
                                          tag="xt", name=f"xt_{q}")
            xt = st[("x", q)]
            nc.sync.dma_start(out=xt[:, jq * T_HALF:(jq + 2) * T_HALF],
                              in_=xin.ap()[:, j:j + 2, :])

        def conv_pair(i):
            """Conv for images i, i+1 (same sample) into one shared 4-bank
            PSUM tile: m-outer so each weight load serves 4 consecutive
            matmuls, striding across all 4 banks so the accumulate
            read-modify-write pipelines."""
            s = i // SENSORS
            if ("ps2", i) not in st:
                st[("ps2", i)] = pspool.tile([OUT_C, 2 * T_OUT], F32,
                                             tag="ps2", name=f"ps2_{i}")
            ps2 = st[("ps2", i)]
            for m in range(3):
                K = 2 * IN_C if m < 2 else IN_C
                w_ap = wt[0:K, (s * 3 + m) * OUT_C:(s * 3 + m + 1) * OUT_C]
                for d in range(2):
                    j = (i + d) % QB
                    xt = st[("x", (i + d) // QB)]
                    for h in range(2):
                        u0 = j * T_HALF + m + h * 512
                        rhs = xt[0:K, u0:u0 + 512]
                        o0 = d * T_OUT + h * 512
                        nc.tensor.matmul(ps2[:, o0:o0 + 512],
                                         w_ap, rhs,
                                         start=(m == 0), stop=(m == 2))

        def relu_pair(i):
            """y = relu(ps + bias) (bf16) for the pair in one ACT."""
            y2 = ypool.tile([OUT_C, 2 * T_OUT], BF16, tag="y2",
                            name=f"y2_{i}")
            nc.scalar.activation(y2[:], st.pop(("ps2", i))[:], AF.Relu,
                                 bias=bias_ap, scale=1.0)
            st[("y", i)] = y2[:, 0:T_OUT]
            st[("y", i + 1)] = y2[:, T_OUT:2 * T_OUT]

        def sblock(i):
            b = i // BLK
            S0 = spool.tile([OUT_C, 32], F32, tag="S", name=f"S_{b}")
            nc.gpsimd.memset(S0[:, 2 * BLK:32], 0.0)
            blk[("S", b)] = S0

        def zsq(i):
            """per-channel mean/var via DVE bn_stats; bn_aggr writes the
            pair strided into the block-stats tile (mean -> col sl,
            var -> col 8+sl)."""
            b, sl = i // BLK, i % BLK
            S = blk[("S", b)]
            y = st[("y", i)]
            bnraw = zpool.tile([OUT_C, 12], F32, tag="bnraw",
                               name=f"bnraw_{i}")
            for h in range(2):
                nc.vector.bn_stats(bnraw[:, 6 * h:6 * h + 6],
                                   y[:, h * 512:(h + 1) * 512])
            nc.vector.bn_aggr(S[:, sl:sl + BLK + 1:BLK], bnraw[:])

        def stats_a(b):
            """Batched GroupNorm stats phase A (all DVE + one ACT sqrt).

            Cross-partition group reduce via two DVE 32x32 block
            transposes (cols 0:BLK means, BLK:2*BLK vars; rest of S is
            never-read garbage that stays confined to unused lanes).
            Keeping the chain on one engine avoids queue-head stalls
            ping-ponging between DVE/Pool/ACT.
            """
            S = blk.pop(("S", b))
            # E2 = var + mean^2 (batched for the BLK images)
            m2 = tpool.tile([OUT_C, BLK], F32, tag="m2", name=f"m2_{b}")
            nc.vector.tensor_mul(m2[:], S[:, 0:BLK], S[:, 0:BLK])
            nc.vector.tensor_add(S[:, BLK:2 * BLK], S[:, BLK:2 * BLK], m2[:])
            tr = tpool.tile([OUT_C, 32], F32, tag="tr", name=f"tr_{b}")
            nc.vector.transpose(tr[:], S[:])
            red = tpool.tile([OUT_C, 1], F32, tag="red", name=f"red_{b}")
            nc.vector.reduce_sum(red[:], tr[:], axis=mybir.AxisListType.X)
            bc = tpool.tile([OUT_C, 32], F32, tag="bc", name=f"bc_{b}")
            nc.vector.tensor_scalar_mul(bc[:], ones32[:], red[:])
            tr2 = tpool.tile([OUT_C, 32], F32, tag="tr2", name=f"tr2_{b}")
            nc.vector.transpose(tr2[:], bc[:])
            su = tr2[:, 0:BLK]        # sum over group of mean_c
            qu = tr2[:, BLK:2 * BLK]  # sum over group of E2_c
            # scale = gamma/sqrt(var+eps); offset = beta - mean*scale
            # via c = G*qu - su^2 = G^2*var; std*G = sqrt(c + G^2*eps)
            a = tpool.tile([OUT_C, BLK], F32, tag="a", name=f"a_{b}")
            nc.vector.tensor_mul(a[:], su, su)
            c = tpool.tile([OUT_C, BLK], F32, tag="c", name=f"c_{b}")
            nc.vector.scalar_tensor_tensor(c[:], qu, float(CPG), a[:],
                                           op0=ALU.mult, op1=ALU.subtract)
            sd = tpool.tile([OUT_C, BLK], F32, tag="sd", name=f"sd_{b}")
            nc.scalar.activation(sd[:], c[:], AF.Sqrt, bias=m2eps_ap)
            blk[("tr2", b)] = tr2
            blk[("sd", b)] = sd

        def stats_b(b):
            """Phase B (issued one image later so the DVE reciprocal never
            waits on the ACT sqrt round trip): scale/offset, all DVE."""
            tr2 = blk.pop(("tr2", b))
            sd = blk.pop(("sd", b))
            su = tr2[:, 0:BLK]
            r = tpool.tile([OUT_C, BLK], F32, tag="r", name=f"r_{b}")
            nc.vector.reciprocal(r[:], sd[:])
            so = spool.tile([OUT_C, 2 * BLK], F32, tag="so", name=f"so_{b}")
            nc.vector.tensor_scalar_mul(so[:, 0:BLK], r[:], mgam_ap)
            t1 = tpool.tile([OUT_C, BLK], F32, tag="t1", name=f"t1_{b}")
            nc.vector.tensor_mul(t1[:], su, so[:, 0:BLK])
            nc.vector.tensor_scalar(so[:, BLK:2 * BLK], t1[:], -1.0 / CPG,
                                    beta_ap, op0=ALU.mult, op1=ALU.add)
            blk[("so", b)] = so

        def apply(j):
            """out = y*scale+offset into the 4-image fp16 output tile."""
            b, sl = j // BLK, j % BLK
            q, jq = j // QB, j % QB
            if jq == 0:
                st[("ot", q)] = opool.tile([OUT_C, QB * T_OUT], FP16,
                                           tag="ot", name=f"ot_{q}")
            ot = st[("ot", q)]
            so = blk[("so", b)]
            scl = so[:, sl:sl + 1]
            off = so[:, BLK + sl:BLK + sl + 1]
            y = st.pop(("y", j))
            o0 = jq * T_OUT
            if j >= IMGS - 2 * BLK:
                # last blocks: bn_stats is winding down, DVE has slack - let it carry
                # the apply so the tail drains fast
                nc.scalar.activation(ot[:, o0:o0 + 256], y[:, 0:256],
                                     AF.Identity, bias=off, scale=scl)
                nc.vector.tensor_scalar(ot[:, o0 + 256:o0 + 768],
                                        y[:, 256:768],
                                        scl, off, op0=ALU.mult, op1=ALU.add)
                nc.gpsimd.tensor_scalar(ot[:, o0 + 768:o0 + T_OUT],
                                        y[:, 768:T_OUT],
                                        scl, off, op0=ALU.mult, op1=ALU.add)
            else:
                nc.scalar.activation(ot[:, o0:o0 + 128], y[:, 0:128],
                                     AF.Identity, bias=off, scale=scl)
                nc.gpsimd.tensor_scalar(ot[:, o0 + 128:o0 + T_OUT],
                                        y[:, 128:T_OUT],
                                        scl, off, op0=ALU.mult, op1=ALU.add)

        def out_dma(j):
            """Steady state: one 4-image DMA per batch.  Last batch:
            single-image DMAs so each image ships the moment its apply
            lands."""
            q, jq = j // QB, j % QB
            ot = st[("ot", q)]
            if j >= IMGS - 2 * QB:
                nc.sync.dma_start(out=out.ap()[:, j, :],
                                  in_=ot[:, jq * T_OUT:(jq + 1) * T_OUT])
                if jq == QB - 1:
                    st.pop(("ot", q))
            elif jq == QB - 1:
                st.pop(("ot", q))
                nc.sync.dma_start(out=out.ap()[:, QB * q:QB * q + QB, :],
                                  in_=ot[:])

        in_dma(0)
        # cons is only needed by relu(0); don't let it delay pair 0
        nc.scalar.dma_start(out=ct[:], in_=cons.ap()[:])
        for p in range(1, 4):
            in_dma(p)
        for i in range(IMGS + LAG + 1):
            if i < IMGS:
                if i % 2 == 0 and i + 8 < IMGS:
                    in_dma((i + 8) // 2)
                if i % 2 == 0:
                    conv_pair(i)
                    relu_pair(i)
                if i % BLK == 0:
                    sblock(i)
                zsq(i)
            # stats phases issued 1 and 2 images AFTER the block completes:
            # the DVE chain and ACT sqrt get slack so no queue-head stalls
            if i % BLK == 0 and BLK <= i <= IMGS:
                stats_a(i // BLK - 1)
            if i % BLK == 1 and BLK < i <= IMGS + 1:
                stats_b((i - 1) // BLK - 1)
            jj = i - LAG
            if 0 <= jj < IMGS:
                apply(jj)
                out_dma(jj)
    nc.compile()
    return nc


def get_program():
    global _PROGRAM
    if _PROGRAM is None:
        _PROGRAM = _build_program()
    return _PROGRAM


def _host_prep(x, A_flat, B_flat, conv_w, conv_b, gamma, beta, num_sensors, r,
               lora_scale):
    x = np.asarray(x, dtype=np.float32)
    A_flat = np.asarray(A_flat, dtype=np.float32)
    B_flat = np.asarray(B_flat, dtype=np.float32)
    conv_w = np.asarray(conv_w, dtype=np.float32)
    conv_b = np.asarray(conv_b, dtype=np.float32)
    gamma = np.asarray(gamma, dtype=np.float32)
    beta = np.asarray(beta, dtype=np.float32)
    batch = A_flat.shape[0]
    out_c, in_c, k = conv_w.shape
    ns = int(num_sensors)
    rr = int(r)
    ls = float(lora_scale)
    assert (batch, out_c, in_c, k) == (32, OUT_C, IN_C, KTAPS)
    assert ns == SENSORS and x.shape == (batch * ns, in_c, T)

    # per-sample effective weight, transposed for the PE (lhsT layout)
    A = A_flat.reshape(batch, rr, in_c * k)
    Bm = B_flat.reshape(batch, out_c, rr)
    delta = np.einsum("bor,brm->bom", Bm, A) * ls
    W = conv_w.reshape(1, out_c, in_c * k) + delta            # (B, out_c, in_c*k)
    WT = W.reshape(batch, out_c, in_c, k).transpose(0, 2, 3, 1)  # (B, ci, k, co)
    # pack tap pairs on the partition axis: tile m rows = [W_T[:, 2m], W_T[:, 2m+1]]
    Wt = np.zeros((batch, 2 * in_c, 3 * out_c), dtype=np.float32)
    for m in range(3):
        Wt[:, 0:in_c, m * out_c:(m + 1) * out_c] = WT[:, :, 2 * m, :]
        if 2 * m + 1 < k:
            Wt[:, in_c:2 * in_c, m * out_c:(m + 1) * out_c] = WT[:, :, 2 * m + 1, :]

    import ml_dtypes
    np_in_dt = ml_dtypes.bfloat16
    # deinterleaved, padded, image-inner: [ci, n, u] = x_pad[n, ci, 2u];
    # [64+ci, n, u] = x_pad[n, ci, 2u+1]
    x_pad = np.zeros((2 * in_c, batch * ns, T_HALF), dtype=np_in_dt)
    x_pad[0:in_c, :, 1:1 + T // 2] = x[:, :, 0::2].transpose(1, 0, 2)
    x_pad[in_c:2 * in_c, :, 1:1 + T // 2] = x[:, :, 1::2].transpose(1, 0, 2)

    cons = np.ascontiguousarray(
        np.stack([conv_b, CPG * gamma, beta,
                  np.full_like(conv_b, CPG * CPG * EPS)], axis=1),
        dtype=np.float32)
    in_maps = []
    for c in range(N_CORES):
        wc = Wt[c * SAMPLES:(c + 1) * SAMPLES]          # (4, 128, 384)
        wc = np.ascontiguousarray(
            wc.transpose(1, 0, 2).reshape(2 * in_c, SAMPLES * 3 * out_c),
            dtype=np_in_dt)
        in_maps.append({
            "xin": np.ascontiguousarray(x_pad[:, c * IMGS:(c + 1) * IMGS]),
            "wts": wc,
            "cons": cons,
        })
    return in_maps


def _maybe_reset_devices():
    """Best-effort NRT reset (recovers a wedged core from a prior crash)."""
    try:
        import ctypes
        lib = ctypes.CDLL("/opt/axon/libaxon_pjrt.so")
        lib.axon_reset.restype = ctypes.c_int64
        lib.axon_reset()
    except Exception:
        pass


def kernel(x, A_flat, B_flat, conv_w, conv_b, gamma, beta, num_sensors, r,
           lora_scale):
    global LAST_RESULTS
    _maybe_reset_devices()
    in_maps = _host_prep(x, A_flat, B_flat, conv_w, conv_b, gamma, beta,
                         num_sensors, r, lora_scale)
    nc = get_program()
    res = run_bass_kernel_spmd(nc, in_maps, core_ids=list(range(N_CORES)),
                               trace=TRACE)
    LAST_RESULTS = res
    full = np.concatenate([res.results[c]["out"] for c in range(N_CORES)],
                          axis=1)                      # (OUT_C, 256, T_OUT)
    return np.ascontiguousarray(full.transpose(1, 0, 2), dtype=np.float32)
